# revision 50
# baseline (speedup 1.0000x reference)
"""Multi-head attention (B=2, S=2048, D=1024, H=16) on 8 TRN2 NeuronCores.

Sharding: core = (batch b, head-group g): 2 batches x 4 groups of 4 heads.
Each core computes its group's QKV projections, attention, and a partial
output projection; the host sums the 4 partials per batch and adds the
exact bias constant (bv @ Wo.T + bo). bq/bk are applied on device.

v2 architecture (slot pipeline, ACT-engine-bound):
  The scalar (ACT) engine's exp stream is the hard floor (~128 chunks of
  [128,1024] at ~1.15us each). Everything is organized so ACT streams exp
  back-to-back while the tensor engine (PE) never idles >3.4us (which
  would re-throttle the HAM clock gate to 1.2 GHz -- the baseline's
  failure mode: 325us of the 423us kernel ran at half PE clock).

  - Attention runs over 128 flat "slots" = (query-half qh, head h, seq
    tile t). Slot g: scores matmuls (PE) + exp (ACT) for slot g, plus PV
    matmuls (PE) for slot g-LAG. PV consumes exp output LAG slots late so
    it is never gated by ACT.
  - ACT runs ONLY exp. Biases ride DVE tensor_scalar (Q scale folded into
    Wq host-side), PSUM evictions ride DVE, softmax reciprocal uses
    reciprocal_approx_fast (DVE, ~5x faster than InstReciprocal).
  - PE filler with deadlines keeps the PE dense: V projection in the
    first 16 slots, Q-proj j1 in head 1's window, out-proj for query-half
    0 during half 1's windows.
  - PSUM: one shared 3-slot ring (6 banks, tag "sp") serves scores / V
    proj / Q-proj j1 / denominator-broadcast / out-proj tiles; 2 banks
    hold the PV accumulator ([128, 1024]: rows 0-63 numerator, row 64
    softmax denominator via a ones-column in the V' stationary).
  - fp16 matmuls (fp32 PSUM accumulation); fp16 DRAM output summed in
    f32 on the host.
"""
from contextlib import ExitStack

import numpy as np

# Problem constants (hardcoded per harness contract).
B, S, D, H = 2, 2048, 1024, 16
HD = D // H          # 64
N_CORES = 8
GROUPS = N_CORES // B    # 4
H_LOC = H // GROUPS      # 4 heads per core
JJ = H_LOC * HD          # 256
P = 128

MM_DT = "fp16"  # "fp16" | "bf16"


def build_mha(s=S, d=D, h_loc=H_LOC, hd=HD, chunk=None, nf=512, mm_dt=MM_DT):
    """Build + compile the per-core Bass program."""
    import concourse.bacc as bacc
    import concourse.tile as tile
    from concourse import mybir

    f32 = mybir.dt.float32
    f32r = mybir.dt.float32r
    _two_byte = {"bf16": mybir.dt.bfloat16, "fp16": mybir.dt.float16}
    mdt = _two_byte[mm_dt]
    Exp = mybir.ActivationFunctionType.Exp
    Add = mybir.AluOpType.add

    jj = h_loc * hd
    hd1 = hd + 1
    ktd = d // P             # contraction tiles of the model dim
    njt = (jj + P - 1) // P  # head-group tiles (KT/QT/OT partition tiles)
    st_n = s // P            # sequence tiles (128 keys each)
    qw = s // 2              # query-half width
    nf = min(nf, qw)         # matmul free-dim chunk (PSUM bank = 512 f32)
    nfc = qw // nf           # matmuls per scores/pv chunk
    ndo = (d + nf - 1) // nf
    LAG = min(8, st_n)       # pv trails scores by LAG slots

    nc = bacc.Bacc("TRN2", target_bir_lowering=False, debug=False)

    xq = nc.dram_tensor("xq", [d, s], mdt, kind="ExternalInput").ap()
    xk = nc.dram_tensor("xk", [d, s], mdt, kind="ExternalInput").ap()
    xv = nc.dram_tensor("xv", [d, s], mdt, kind="ExternalInput").ap()
    wq = nc.dram_tensor("wq", [d, jj], mdt, kind="ExternalInput").ap()
    wk = nc.dram_tensor("wk", [d, jj], mdt, kind="ExternalInput").ap()
    wv = nc.dram_tensor("wv", [d, jj], mdt, kind="ExternalInput").ap()
    wo = nc.dram_tensor("wo", [jj, d], mdt, kind="ExternalInput").ap()
    bqp = nc.dram_tensor("bqp", [jj, 1], f32, kind="ExternalInput").ap()
    bkp = nc.dram_tensor("bkp", [jj, 1], f32, kind="ExternalInput").ap()
    out = nc.dram_tensor("out", [s, d], mdt, kind="ExternalOutput").ap()

    # x loads ride sync+gpsimd ONLY: each DGE ring holds ~16 in-flight
    # descriptors and a dma_start past that depth blocks the issuing
    # engine's FIFO -- putting bulk loads on the scalar queue would stall
    # the exp stream behind the ring. Priority comes from chunking: a
    # tensor's chunks fill the ring and later tensors queue behind.
    def xqueues():
        engs = [nc.sync, nc.gpsimd]
        i = 0
        while True:
            yield engs[i % len(engs)]
            i += 1

    xq_rr = xqueues()
    out_rr_engs = [nc.sync, nc.gpsimd]

    with tile.TileContext(nc) as tc, ExitStack() as ctx:
        persist = ctx.enter_context(tc.tile_pool(name="persist", bufs=1))

        qt_sb = [persist.tile([P, s], mdt, name=f"qt{j}", tag=f"qt{j}") for j in range(njt)]
        kt_sb = [persist.tile([P, s], mdt, name=f"kt{j}", tag=f"kt{j}") for j in range(njt)]
        ot_sb = [persist.tile([P, s], mdt, name=f"ot{j}", tag=f"ot{j}") for j in range(njt)]
        # padded per-(seq-tile, head) PV stationaries: [V_h | ones | zeros]
        v_sb = [[persist.tile([P, P], mdt, name=f"v{t}_{h}", tag=f"v{t}_{h}")
                 for h in range(h_loc)] for t in range(st_n)]
        wq_r = [persist.tile([P, jj], mdt, name=f"wqr{k}", tag=f"wqr{k}") for k in range(ktd)]
        wk_r = [persist.tile([P, jj], mdt, name=f"wkr{k}", tag=f"wkr{k}") for k in range(ktd)]
        wv_r = [persist.tile([P, jj], mdt, name=f"wvr{k}", tag=f"wvr{k}") for k in range(ktd)]
        wo_r = [persist.tile([P, d], mdt, name=f"wor{j}", tag=f"wor{j}") for j in range(njt)]
        bq_sb = persist.tile([P, njt], f32, name="bq_sb", tag="bq_sb")
        bk_sb = persist.tile([P, njt], f32, name="bk_sb", tag="bk_sb")
        ones_v = persist.tile([P, 1], f32, name="ones_v", tag="ones_v")
        ones_hm = persist.tile([1, hd], mdt, name="ones_hm", tag="ones_hm")
        dum = persist.tile([1, 2], f32, name="dum", tag="dum")
        dum2 = persist.tile([1, 2], f32, name="dum2", tag="dum2")

        # preload the exp ACT table set while DMAs run
        nc.vector.memset(dum[:], 0.0)
        nc.scalar.activation(dum2[:], dum[:], Exp)
        nc.vector.memset(ones_v[:], 1.0)
        nc.vector.memset(ones_hm[:], 1.0)

        # ---- DMA issue, ordered along the lead-in critical path. x
        # tensors are column-split (halves; xv in quarters) so consumers
        # with partial-column needs start as early as possible:
        #   wk | xk_lo | wq | xq_lo -> first scores at ~xq_lo landing
        #   xk_hi (scores t>=8) | wv | xv quarters (V proj slots) | xq_hi
        hw = s // 2
        qtr = max(P, s // 4)
        nqtr = s // qtr

        def half_loads(pool, dram, nm):
            lo, hi = [], []
            for k in range(ktd):
                xt = pool.tile([P, hw], mdt, name=f"{nm}l{k}", tag=f"{nm}l{k}")
                lo.append(xt)
            for k in range(ktd):
                xt = pool.tile([P, hw], mdt, name=f"{nm}h{k}", tag=f"{nm}h{k}")
                hi.append(xt)
            return lo, hi

        xk_pool = ctx.enter_context(tc.tile_pool(name="xkpool", bufs=1))
        xq_pool = ctx.enter_context(tc.tile_pool(name="xqpool", bufs=1))
        xv_pool = ctx.enter_context(tc.tile_pool(name="xvpool", bufs=1))
        xk_lo, xk_hi = half_loads(xk_pool, xk, "xk")
        xq_lo, xq_hi = half_loads(xq_pool, xq, "xq")
        xv_t = [[xv_pool.tile([P, qtr], mdt, name=f"xv{q}_{k}", tag=f"xv{q}_{k}")
                 for k in range(ktd)] for q in range(nqtr)]

        # Each DGE queue fair-shares among its (up to ~16) in-flight
        # descriptors, so issue order alone gives no priority. Split every
        # x transfer into column chunks: earlier tensors then fill the
        # in-flight window and later ones genuinely queue behind.
        CH = max(P, hw // 2)   # 512-col chunks (128KB)

        def chunked(dst, dram_row, c0, c1, ch=None):
            ch = ch or CH
            for x0 in range(c0, c1, ch):
                x1 = min(x0 + ch, c1)
                next(xq_rr).dma_start(dst[:, x0 - c0:x1 - c0],
                                      dram_row[:, x0:x1])

        for k in range(ktd):
            nc.gpsimd.dma_start(wk_r[k][:], wk[k * P:(k + 1) * P, :])
            nc.gpsimd.dma_start(wq_r[k][:], wq[k * P:(k + 1) * P, :])
            nc.gpsimd.dma_start(wv_r[k][:], wv[k * P:(k + 1) * P, :])
        for j in range(njt):
            nc.gpsimd.dma_start(bq_sb[:, j:j + 1], bqp[j * P:(j + 1) * P, :])
            nc.gpsimd.dma_start(bk_sb[:, j:j + 1], bkp[j * P:(j + 1) * P, :])
        # first-needed tensors in finer chunks: the DGE ring fair-shares
        # among ~16 in-flight descriptors, so 4 chunks/tile means the
        # earliest k-tiles complete first and projections stream behind
        for k in range(ktd):
            chunked(xk_lo[k], xk[k * P:(k + 1) * P, :], 0, hw)
        for k in range(ktd):
            chunked(xq_lo[k], xq[k * P:(k + 1) * P, :], 0, hw)
        for k in range(ktd):
            chunked(xk_hi[k], xk[k * P:(k + 1) * P, :], hw, s)
        for q in range(nqtr):
            for k in range(ktd):
                chunked(xv_t[q][k], xv[k * P:(k + 1) * P, :],
                        q * qtr, (q + 1) * qtr)
        for k in range(ktd):
            chunked(xq_hi[k], xq[k * P:(k + 1) * P, :], hw, s)
        for j in range(njt):
            nc.gpsimd.dma_start(wo_r[j][:], wo[j * P:(j + 1) * P, :])

        # shared PSUM rings: "sp" ring (3 x [128, qw] f32 slots) serves
        # scores / V-proj / Q-proj-j1 / denom-broadcast / out-proj tiles;
        # "otp" holds the PV accumulator.
        spsum = ctx.enter_context(tc.tile_pool(name="spsum", bufs=3, space="PSUM"))
        opsum = ctx.enter_context(tc.tile_pool(name="opsum", bufs=1, space="PSUM"))
        ptpool = ctx.enter_context(tc.tile_pool(name="ptpool", bufs=LAG + 7))
        npool = ctx.enter_context(tc.tile_pool(name="npool", bufs=2))
        fout = ctx.enter_context(tc.tile_pool(name="fout", bufs=3))

        def make_proj_halves(x_t, xbase, w_r, dst, bias_sb, j, c0, c1, nm):
            """Projection block dst[j][:, c0*nf:c1*nf] = w.T @ x + bias as
            two ~1.7us half-closures (k 0..ktd/2, then the rest + bias).
            x_t are column-split buffers starting at column xbase."""
            width = (c1 - c0) * nf
            nch = c1 - c0
            # psum accumulation groups are per 2KB zero-region: start/stop
            # only on the first/last matmul touching each region
            cpr = max(1, 2048 // (nf * 4))
            st = {}

            def emit(kr0, kr1):
                for k in range(kr0, kr1):
                    for c in range(nch):
                        x0 = (c0 + c) * nf - xbase
                        nc.tensor.matmul(
                            st["pp"][:, c * nf:(c + 1) * nf],
                            w_r[k][:, j * P:(j + 1) * P],
                            x_t[k][:, x0:x0 + nf],
                            start=(k == 0 and c % cpr == 0),
                            stop=(k == ktd - 1
                                  and (c % cpr == cpr - 1 or c == nch - 1)))

            def half0():
                st["pp"] = spsum.tile([P, width], f32, name=nm, tag="sp")
                emit(0, ktd // 2)

            def half1():
                emit(ktd // 2, ktd)
                nc.vector.tensor_scalar(
                    dst[j][:, c0 * nf:c1 * nf], st["pp"][:],
                    bias_sb[:, j:j + 1], None, Add)

            return half0, half1

        def proj_block(x_t, xbase, w_r, dst, bias_sb, j, c0, c1, nm):
            h0_, h1_ = make_proj_halves(x_t, xbase, w_r, dst, bias_sb,
                                        j, c0, c1, nm)
            h0_()
            h1_()

        # ---- PE warm-up spin: ~26 dependency-free matmuls un-throttle the
        # HAM clock gate (cold 1.2 GHz -> warm 2.4 GHz) while the x DMAs
        # land, so the DMA-paced projections run at full clock ----
        if njt > 1:
            dw = persist.tile([P, P], mdt, name="dw", tag="dw")
            dm = persist.tile([P, nf], mdt, name="dm", tag="dm")
            nc.vector.memset(dw[:], 0.25)
            nc.vector.memset(dm[:], 0.25)
            for i in range(26):
                dps = spsum.tile([P, nf], f32, name=f"dps{i}", tag="sp")
                nc.tensor.matmul(dps[:], dw[:], dm[:], start=True, stop=True)

        # ---- lead-in: only what the first scores need (K/Q j0, low cols) --
        spc = s // nf
        spc2 = max(1, spc // 2)
        proj_block(xk_lo, 0, wk_r, kt_sb, bk_sb, 0, 0,
                   1 if njt > 1 else spc2, "ppk0a")
        proj_block(xq_lo, 0, wq_r, qt_sb, bq_sb, 0, 0, spc2, "ppq0a")
        if njt == 1:
            # small config: the high-column blocks are needed almost
            # immediately; emit them in the lead-in
            proj_block(xk_hi, hw, wk_r, kt_sb, bk_sb, 0, spc2, spc, "ppk0b")
            proj_block(xq_hi, hw, wq_r, qt_sb, bq_sb, 0, spc2, spc, "ppq0b")

        # ---- attention slot pipeline ----
        slots = [(qh, h, t) for qh in range(2) for h in range(h_loc)
                 for t in range(st_n)]
        n_slots = len(slots)
        pts = {}
        otps = {}
        # deferred closures: norms are priority (their npool tiles gate the
        # next-but-one evict on the DVE FIFO -- draining one later than that
        # deadlocks); outproj units are background filler
        fill_norms = []   # [(push_slot, closure)]
        fill_rest = []

        def v_proj_tile(t):
            q = (t * P) // qtr
            tc0 = t * P - q * qtr
            pv_ps = spsum.tile([P, jj], f32, name=f"vps{t}", tag="sp")
            for k in range(ktd):
                nc.tensor.matmul(pv_ps[:], xv_t[q][k][:, tc0:tc0 + P],
                                 wv_r[k][:], start=(k == 0), stop=(k == ktd - 1))
            for h in range(h_loc):
                vt = v_sb[t][h]
                nc.vector.tensor_copy(vt[:, 0:hd], pv_ps[:, h * hd:(h + 1) * hd])
                nc.vector.tensor_copy(vt[:, hd:hd1], ones_v[:])
                if hd1 < P:
                    nc.gpsimd.memset(vt[:, hd1:P], 0.0)

        def scores_unit(qh, h, t):
            jt = (h * hd) // P
            off = (h * hd) % P
            sp = spsum.tile([P, qw], f32, name=f"sp{qh}_{h}_{t}", tag="sp")
            for f in range(nfc):
                q0 = qh * qw + f * nf
                nc.tensor.matmul(
                    sp[:, f * nf:(f + 1) * nf],
                    kt_sb[jt][off:off + hd, t * P:(t + 1) * P],
                    qt_sb[jt][off:off + hd, q0:q0 + nf],
                    start=True, stop=True)
            pt = ptpool.tile([P, qw], mdt, name=f"pt{qh}_{h}_{t}", tag="pt")
            nc.scalar.activation(pt[:], sp[:], Exp)
            pts[qh, h, t] = pt

        def norm_unit(qh, h, rs_r, ob):
            jt = (h * hd) // P
            off = (h * hd) % P
            last = qh == 1 and h == h_loc - 1
            # the very last norm gates the tail out-projection: chunk it
            # finely and emit each pair of out-proj tiles eagerly
            cw = nf if last else qw
            for e in range(qw // cw):
                bp = spsum.tile([hd, cw], f32, name=f"bp{qh}_{h}_{e}", tag="sp")
                for f in range(cw // nf):
                    q0 = e * cw + f * nf
                    nc.tensor.matmul(bp[:, f * nf:(f + 1) * nf],
                                     ones_hm[:], rs_r[:, q0:q0 + nf],
                                     start=True, stop=True)
                # reciprocal in-place in PSUM (saves an SBUF ring)
                nc.vector.reciprocal_approx_fast(out=bp[:], in_=bp[:])
                nc.vector.tensor_mul(
                    ot_sb[jt][off:off + hd,
                              qh * qw + e * cw:qh * qw + (e + 1) * cw],
                    ob[:, e * cw:(e + 1) * cw], bp[:])
                if last:
                    t0 = qh * (st_n // 2) + e * (cw // P)
                    for t in range(t0, t0 + cw // P):
                        outproj_unit(t)
            if h == h_loc - 1 and not last:
                t0 = qh * (st_n // 2)
                for t in range(t0, t0 + st_n // 2):
                    fill_rest.append(lambda t=t: outproj_unit(t))

        def outproj_unit(t):
            po = spsum.tile([P, d], f32, name=f"po{t}", tag="sp")
            for njx in range(ndo):
                for j in range(njt):
                    nc.tensor.matmul(
                        po[:, njx * nf:(njx + 1) * nf],
                        ot_sb[j][:, t * P:(t + 1) * P],
                        wo_r[j][:, njx * nf:(njx + 1) * nf],
                        start=(j == 0), stop=(j == njt - 1))
            ob2 = fout.tile([P, d], mdt, name=f"fo{t}", tag="fo")
            d2 = d // 2
            for hf in range(2):
                nc.vector.tensor_copy(ob2[:, hf * d2:(hf + 1) * d2],
                                      po[:, hf * d2:(hf + 1) * d2])
                out_rr_engs[(t + hf) % 2].dma_start(
                    out[t * P:(t + 1) * P, hf * d2:(hf + 1) * d2],
                    ob2[:, hf * d2:(hf + 1) * d2])

        def pv_unit(qh, h, t):
            if t == 0:
                otps[qh, h] = opsum.tile([P, qw], f32, name=f"otp{qh}_{h}",
                                         tag="otp")
            otp = otps[qh, h]
            pt = pts.pop((qh, h, t))
            for f in range(nfc):
                nc.tensor.matmul(
                    otp[:, f * nf:(f + 1) * nf], v_sb[t][h][:],
                    pt[:, f * nf:(f + 1) * nf],
                    start=(t == 0), stop=(t == st_n - 1))
            if t == st_n - 1:
                # evict: denominator row -> rs (fp16), numerator -> ob (f32)
                rs_r = npool.tile([1, qw], mdt, name=f"rs{qh}_{h}", tag="rs")
                nc.vector.tensor_copy(rs_r[:], otp[hd:hd1, :])
                ob = npool.tile([hd, qw], f32, name=f"ob{qh}_{h}", tag="ob")
                nc.vector.tensor_copy(ob[:], otp[0:hd, :])
                del otps[qh, h]
                fill_norms.append((cur_g[0], lambda: norm_unit(qh, h, rs_r, ob)))

        # ---- inline-filler schedule: slot g -> closures, placed ahead of
        # their consumers (deadlines in comments are slot indices) ----
        slot_extra = {}

        def sched(g, *closures):
            slot_extra.setdefault(g, []).extend(closures)

        # V projection tile t: placed after its xv quarter's expected DMA
        # landing; consumed by pv at slot t + h0's pv lag
        v_off = 8 if LAG >= 8 else max(LAG - 1, 1)
        for t in range(st_n):
            sched(t + v_off, lambda t=t: v_proj_tile(t))
        if njt > 1:
            # full-size schedule (h_loc=4, st_n=16): windows h1+ carry the
            # remaining projections as single-chunk blocks (~1.7us each;
            # never hold a PSUM ring slot across a slot boundary).
            # deadline = consumer slot (in comments)
            def pb(x_t, xb, w_r, dst, b_sb, j, c, nm):
                return lambda: proj_block(x_t, xb, w_r, dst, b_sb,
                                          j, c, c + 1, nm)

            sched(0, pb(xk_lo, 0, wk_r, kt_sb, bk_sb, 0, 1, "ppk0a1"))  # 4
            sched(2, pb(xk_hi, hw, wk_r, kt_sb, bk_sb, 0, 2, "ppk0b0"))
            sched(4, pb(xk_hi, hw, wk_r, kt_sb, bk_sb, 0, 3, "ppk0b1"))  # 8
            sched(20, pb(xk_lo, 0, wk_r, kt_sb, bk_sb, 1, 0, "ppk1a0"))
            sched(22, pb(xk_lo, 0, wk_r, kt_sb, bk_sb, 1, 1, "ppk1a1"))  # 32
            sched(24, pb(xk_hi, hw, wk_r, kt_sb, bk_sb, 1, 2, "ppk1b0"))
            sched(26, pb(xk_hi, hw, wk_r, kt_sb, bk_sb, 1, 3, "ppk1b1"))  # 40
            sched(27, pb(xq_lo, 0, wq_r, qt_sb, bq_sb, 1, 0, "ppq1a0"))
            sched(29, pb(xq_lo, 0, wq_r, qt_sb, bq_sb, 1, 1, "ppq1a1"))  # 32
            sched(40, pb(xq_hi, hw, wq_r, qt_sb, bq_sb, 0, 2, "ppq0b0"))
            sched(42, pb(xq_hi, hw, wq_r, qt_sb, bq_sb, 0, 3, "ppq0b1"))  # 64
            sched(44, pb(xq_hi, hw, wq_r, qt_sb, bq_sb, 1, 2, "ppq1b0"))
            sched(46, pb(xq_hi, hw, wq_r, qt_sb, bq_sb, 1, 3, "ppq1b1"))  # 96

        def pv_lag(idx):
            pqh, ph, _ = slots[idx]
            if pqh == 0 and ph == 0 and LAG >= 8:
                return LAG + 4     # h0 waits on xv quarters landing
            # shorter lag for the final phase trims the serial tail
            return min(4, LAG) if (pqh == 1 and ph == h_loc - 1) else LAG

        pv_cur = 0
        cur_g = [0]
        for g, (qh, h, t) in enumerate(slots):
            cur_g[0] = g
            scores_unit(qh, h, t)
            for cl in slot_extra.get(g, []):
                cl()
            while pv_cur < n_slots and pv_cur + pv_lag(pv_cur) <= g:
                pv_unit(*slots[pv_cur])
                pv_cur += 1
            # PE spare budget is ~0.3us/slot; closures are 1-2us -> 1 per 6
            # slots. Norms must drain within ~12 slots of their push (their
            # npool slots gate evicts two head-boundaries later).
            drainable = g % 6 == 5 and g not in slot_extra
            if fill_norms and (drainable or g - fill_norms[0][0] > 10):
                fill_norms.pop(0)[1]()
            elif fill_rest and drainable:
                fill_rest.pop(0)()
        while pv_cur < n_slots:
            cur_g[0] = n_slots + (pv_cur - (n_slots - LAG))
            pv_unit(*slots[pv_cur])
            pv_cur += 1
            if fill_norms:
                fill_norms.pop(0)[1]()
            elif fill_rest:
                fill_rest.pop(0)()
        while fill_norms or fill_rest:
            if fill_norms:
                fill_norms.pop(0)[1]()
            else:
                fill_rest.pop(0)()

    nc.compile()
    return nc


_NC_CACHE = {}


def _get_nc():
    key = MM_DT
    if key not in _NC_CACHE:
        _NC_CACHE[key] = build_mha(mm_dt=key)
    return _NC_CACHE[key]


def build_in_maps(inputs, mm_dt=MM_DT):
    if mm_dt == "bf16":
        import ml_dtypes
        xdt = ml_dtypes.bfloat16
    else:
        xdt = np.float16

    q = np.asarray(inputs["query"], np.float32)
    k = np.asarray(inputs.get("key_", inputs.get("key")), np.float32)
    v = np.asarray(inputs["value"], np.float32)
    Wq = np.asarray(inputs["Wq"], np.float32)
    Wk = np.asarray(inputs["Wk"], np.float32)
    Wv = np.asarray(inputs["Wv"], np.float32)
    Wo = np.asarray(inputs["Wo"], np.float32)
    bq = np.asarray(inputs["bq"], np.float32)
    bk = np.asarray(inputs["bk"], np.float32)

    sc = np.float32(1.0 / np.sqrt(HD))
    qT = [np.ascontiguousarray(q[b].T).astype(xdt) for b in range(B)]
    kT = [np.ascontiguousarray(k[b].T).astype(xdt) for b in range(B)]
    vT = [np.ascontiguousarray(v[b].T).astype(xdt) for b in range(B)]
    WqT = np.ascontiguousarray(Wq.T) * sc  # fold the 1/sqrt(hd) scale
    WkT = np.ascontiguousarray(Wk.T)
    WvT = np.ascontiguousarray(Wv.T)

    in_maps = []
    for core in range(N_CORES):
        b, g = divmod(core, GROUPS)
        sl = slice(g * JJ, (g + 1) * JJ)
        in_maps.append({
            "xq": qT[b],
            "xk": kT[b],
            "xv": vT[b],
            "wq": np.ascontiguousarray(WqT[:, sl]).astype(xdt),
            "wk": np.ascontiguousarray(WkT[:, sl]).astype(xdt),
            "wv": np.ascontiguousarray(WvT[:, sl]).astype(xdt),
            "wo": np.ascontiguousarray(Wo[:, sl].T).astype(xdt),
            "bqp": np.ascontiguousarray((bq[sl] * sc)[:, None]),
            "bkp": np.ascontiguousarray(bk[sl][:, None]),
        })
    return in_maps


def combine_outputs(results, inputs):
    Wo = np.asarray(inputs["Wo"], np.float32)
    bv = np.asarray(inputs["bv"], np.float32)
    bo = np.asarray(inputs["bo"], np.float32)
    const = bv @ Wo.T + bo  # exact host-side bias correction
    outp = np.empty((B, S, D), np.float32)
    for b in range(B):
        acc = results[b * GROUPS]["out"].astype(np.float32).copy()
        for g in range(1, GROUPS):
            acc += results[b * GROUPS + g]["out"].astype(np.float32)
        outp[b] = acc + const[None, :]
    return outp


def kernel(**inputs):
    import time
    from concourse.bass_utils import run_bass_kernel_spmd

    nc = _get_nc()
    in_maps = build_in_maps(inputs)
    last_err = None
    for attempt in range(3):
        try:
            res = run_bass_kernel_spmd(nc, in_maps, list(range(N_CORES)))
            return combine_outputs(res.results, inputs)
        except Exception as e:  # transient device wedge: retry
            last_err = e
            try:
                import jax
                import jax.numpy as jnp
                for dvc in jax.devices()[:N_CORES]:
                    jax.device_put(jnp.zeros((8, 8)), dvc).block_until_ready()
            except Exception:
                pass
            time.sleep(5.0 * (attempt + 1))
    raise last_err
